# revision 19
# baseline (speedup 1.0000x reference)
"""Trainium2 Bass kernel for MultiHeadAttention with relative position bias.

Reference computation (B=2, S=2048, D=1024, H=16, Dk=64, MAX_REL=128):
    Q,K,V = x@W{q,k,v}.T + b      (per-head reshape)
    scores = QK^T/sqrt(Dk) + rel_bias_matrix
    out = softmax(scores) @ V, heads merged, @ Wo.T + bo

Sharding (8 cores): core c handles batch b=c//4 and 4 heads hg=4*(c%4)..+4
(data + head parallel). Q/K/V projections column-split per head group,
Wo row-split; the partial outputs are summed on the host (the "all-reduce").

v2 design (all-bf16 matmuls, engine-balanced):
  - All matmul operands bf16 (host converts); PSUM accumulation f32.
  - Projections: xT/W chunked DMAs overlap the first flight matmuls.
  - Attention per (qh, pair): 16 k-chunks; per chunk QK^T (PE) -> exp
    (ACT, the bottleneck engine: only it has exp) -> future-region fixup
    (DVE) + Toeplitz band fixup (Pool) -> PV (PE, one k-chunk behind -
    software pipelined).  Fully-"future" tiles fold the constant fixup
    into the exp bias, skipping DVE entirely.
  - Softmax denominator rides as a ones-column on V.  Reciprocal runs on
    a DMA-transposed [128, 2, 8] layout (128 lanes instead of 1), then a
    stride-0 DMA broadcasts it back across partitions.
  - ACT does nothing but exp during attention; evictions go to Pool/DVE.
"""

import math
import os
import sys

for _p in ("/opt/trn_rl_repo", "/root/.axon_site", "/root/.axon_site/_ro/trn_rl_repo",
           "/root/.axon_site/_ro/pypackages"):
    if os.path.isdir(_p) and _p not in sys.path:
        sys.path.append(_p)

import numpy as np
import ml_dtypes

import concourse.bass as bass
import concourse.mybir as mybir
import concourse.tile as tile
from concourse import bacc
from contextlib import ExitStack

# Problem constants (hardcoded per the contract).
B, S, D = 2, 2048, 1024
H, DK = 16, 64
MAX_REL = 128
N_CORES = 8
CORES_PER_BATCH = 4
HEADS_PER_CORE = H // CORES_PER_BATCH  # 4
CL = HEADS_PER_CORE * DK               # 256 local channels
QH = 1024                              # q processed in halves
N_QH = S // QH                         # 2
N_KC = S // 128                        # 16 k chunks
BAND = 3 * 128                         # band width in q for one k chunk
NDC = D // 128                         # 8 contraction chunks

F32 = mybir.dt.float32
BF16 = mybir.dt.bfloat16
FP8 = mybir.dt.float8e4
WSCALE = 32.0          # host multiplies W by this before fp8 quantization
FW = QH                # constant-multiplier prefix width of the bandx tile

SCALE = 1.0 / math.sqrt(DK)
# constant subtracted inside exp so fp8 P~ never overflows e4m3's max 448;
# cancels exactly in the softmax normalization.
PSHIFT = -3.0
EXP = mybir.ActivationFunctionType.Exp


def build_program():
    nc = bacc.Bacc("TRN2", target_bir_lowering=False, debug=False)

    xt_d = nc.declare_dram_parameter("xt", [D, S], BF16, isOutput=False)
    wqt_d = nc.declare_dram_parameter("wqt", [D, CL], BF16, isOutput=False)
    wkt_d = nc.declare_dram_parameter("wkt", [D, CL], BF16, isOutput=False)
    wvt_d = nc.declare_dram_parameter("wvt", [D, CL], BF16, isOutput=False)
    wot_d = nc.declare_dram_parameter("wot", [CL, D], BF16, isOutput=False)
    bqk_d = nc.declare_dram_parameter("bqk", [128, 4], F32, isOutput=False)
    # bandx[:, h, 0:FW] = exp(c_fut-c_past); [:, h, FW:] = exp(bias-c_past):
    # one strided multiply fixes the future region and the Toeplitz band.
    bandx_d = nc.declare_dram_parameter("bandx", [128, HEADS_PER_CORE, FW + BAND],
                                        BF16, isOutput=False)
    fbias_d = nc.declare_dram_parameter("fbias", [128, HEADS_PER_CORE], F32,
                                        isOutput=False)
    out_d = nc.declare_dram_parameter("out_p", [S, D], BF16, isOutput=True)
    # denominator scratch, row r = (qh*2+pair)*2 + hh
    denr_d = nc.dram_tensor("den_raw", [2 * N_QH * 2, QH], F32)
    denc_d = nc.dram_tensor("den_rec", [2 * N_QH * 2, QH], F32)

    with tile.TileContext(nc) as tc, ExitStack() as ctx:
        # ---------- long-lived SBUF ----------
        persist = ctx.enter_context(tc.tile_pool(name="persist", bufs=1))
        q_sb = persist.tile([128, 2, S], BF16, tag="q_sb")
        k_sb = persist.tile([128, 2, S], BF16, tag="k_sb")
        v_sb = persist.tile([128, N_KC, HEADS_PER_CORE, DK + 1], BF16, tag="v_sb")
        ct_sb = persist.tile([128, 2, S], BF16, tag="ct_sb")
        wo_sb = persist.tile([128, 2, D], BF16, tag="wo_sb")
        bandx_sb = persist.tile([128, HEADS_PER_CORE, FW + BAND], BF16,
                                tag="bandx_sb")
        bqk_sb = persist.tile([128, 4], F32, tag="bqk_sb")
        fbias_sb = persist.tile([128, HEADS_PER_CORE], F32, tag="fbias_sb")
        warm_sb = persist.tile([1, 2], F32, tag="warm_sb")
        pshift_sb = persist.tile([128, 1], F32, tag="pshift_sb")

        # Warm the ACT exp table at t=0 so the first real exp doesn't pay
        # the table load.
        nc.vector.memset(warm_sb[:, 0:1], 0.0)
        nc.vector.memset(pshift_sb, PSHIFT)
        nc.scalar.activation(out=warm_sb[:, 1:2], in_=warm_sb[:, 0:1],
                             func=EXP, scale=1.0)
        # PE warmup: the HAM clock gate keeps the PE at 1.2 GHz until it has
        # been busy ~3.4us. Burn dummy matmuls while the input DMAs stream so
        # the real projections start at full clock.
        wmm_sb = persist.tile([128, 512], BF16, tag="wmm_sb")
        nc.vector.memset(wmm_sb, 0.0)
        # V's ones column (gives the softmax denominator for free).
        nc.gpsimd.memset(v_sb[:, :, :, DK:DK + 1], 1.0)

        # ---------- PSUM pools ----------
        stp = ctx.enter_context(tc.tile_pool(name="stp", bufs=2, space="PSUM"))
        outp = ctx.enter_context(tc.tile_pool(name="outp", bufs=4))

        # ---------- phase 1: input DMAs + projections ----------
        with ExitStack() as proj_ctx:
            wps = stp.tile([128, 512], F32, tag="st")
            for _ in range(16):
                nc.tensor.matmul(wps, lhsT=wmm_sb[:, 0:128], rhs=wmm_sb,
                                 start=True, stop=True)
            paccp = proj_ctx.enter_context(
                tc.tile_pool(name="paccp", bufs=2, space="PSUM"))
            xw = proj_ctx.enter_context(tc.tile_pool(name="xw", bufs=1))
            xt_sb = xw.tile([128, NDC, S], BF16, tag="xt_sb")
            wq_sb = xw.tile([128, NDC, CL], BF16, tag="wq_sb")
            wk_sb = xw.tile([128, NDC, CL], BF16, tag="wk_sb")
            wv_sb = xw.tile([128, NDC, CL], BF16, tag="wv_sb")

            xt_v = xt_d.ap().rearrange("(c p) s -> p c s", p=128)
            wq_v = wqt_d.ap().rearrange("(c p) m -> p c m", p=128)
            wk_v = wkt_d.ap().rearrange("(c p) m -> p c m", p=128)
            wv_v = wvt_d.ap().rearrange("(c p) m -> p c m", p=128)

            # Interleave weight/x chunk DMAs so the first flight group can
            # start after ~0.6MB instead of after all 7MB.
            nc.sync.dma_start(out=wq_sb[:, 0:1, :], in_=wq_v[:, 0:1, :])
            nc.scalar.dma_start(out=xt_sb[:, 0, :], in_=xt_v[:, 0, :])
            nc.sync.dma_start(out=wq_sb[:, 1:2, :], in_=wq_v[:, 1:2, :])
            nc.scalar.dma_start(out=xt_sb[:, 1, :], in_=xt_v[:, 1, :])
            nc.sync.dma_start(out=bqk_sb, in_=bqk_d.ap())
            for g in range(1, 4):
                d0 = 2 * g
                nc.sync.dma_start(out=wq_sb[:, d0:d0 + 2, :], in_=wq_v[:, d0:d0 + 2, :])
                nc.scalar.dma_start(out=xt_sb[:, d0, :], in_=xt_v[:, d0, :])
                nc.scalar.dma_start(out=xt_sb[:, d0 + 1, :], in_=xt_v[:, d0 + 1, :])
                nc.sync.dma_start(out=wk_sb[:, d0 - 2:d0, :], in_=wk_v[:, d0 - 2:d0, :])
                nc.sync.dma_start(out=wv_sb[:, d0 - 2:d0, :], in_=wv_v[:, d0 - 2:d0, :])
            nc.sync.dma_start(out=wk_sb[:, 6:8, :], in_=wk_v[:, 6:8, :])
            nc.sync.dma_start(out=wv_sb[:, 6:8, :], in_=wv_v[:, 6:8, :])
            nc.sync.dma_start(out=fbias_sb, in_=fbias_d.ap())
            nc.scalar.dma_start(out=bandx_sb, in_=bandx_d.ap())
            nc.sync.dma_start(out=wo_sb, in_=wot_d.ap().rearrange("(c p) m -> p c m", p=128))

            GROUPS = [(0,), (1,), (2, 3), (4, 5), (6, 7)]

            def qk_flight(w_sb, o_sb, boff, eng):
                slot0 = stp.tile([128, 1024], F32, tag="st")
                slot1 = stp.tile([128, 1024], F32, tag="st")
                slot2 = paccp.tile([128, 1024], F32, tag="acc")
                slot3 = paccp.tile([128, 1024], F32, tag="acc")
                slots = [slot0, slot1, slot2, slot3]
                for g in GROUPS:
                    for j in range(2):
                        for t in range(2):
                            ps = slots[j * 2 + t]
                            for half in range(2):
                                for dc in g:
                                    nc.tensor.matmul(
                                        ps[:, half * 512:(half + 1) * 512],
                                        lhsT=w_sb[:, dc, j * 128:(j + 1) * 128],
                                        rhs=xt_sb[:, dc, t * 1024 + half * 512:
                                                  t * 1024 + (half + 1) * 512],
                                        start=(dc == 0), stop=(dc == NDC - 1),
                                    )
                for j in range(2):
                    for t in range(2):
                        eng.tensor_scalar_add(
                            out=o_sb[:, j, t * 1024:(t + 1) * 1024],
                            in0=slots[j * 2 + t],
                            scalar1=bqk_sb[:, boff + j:boff + j + 1],
                        )

            qk_flight(wq_sb, q_sb, 0, nc.vector)
            qk_flight(wk_sb, k_sb, 2, nc.vector)

            # V: [s_chunk, dv], 4 s-chunks packed per 2 PSUM slots
            for scg in range(N_KC // 4):
                ps = stp.tile([128, 1024], F32, tag="st")
                psb = paccp.tile([128, 1024], F32, tag="acc")
                both = (ps, psb)
                for g in GROUPS:
                    for i in range(4):
                        sc = scg * 4 + i
                        tgt = both[i // 2]
                        col = (i % 2) * 512
                        for dc in g:
                            nc.tensor.matmul(
                                tgt[:, col:col + CL],
                                lhsT=xt_sb[:, dc, sc * 128:(sc + 1) * 128],
                                rhs=wv_sb[:, dc, :],
                                start=(dc == 0), stop=(dc == NDC - 1),
                            )
                for i in range(4):
                    sc = scg * 4 + i
                    tgt = both[i // 2]
                    col = (i % 2) * 512
                    nc.scalar.copy(
                        out=v_sb[:, sc, :, 0:DK],
                        in_=tgt[:, col:col + CL].rearrange("p (h d) -> p h d",
                                                           h=HEADS_PER_CORE),
                    )

        # ---------- phase 2: attention ----------
        # P~ = exp(s/8)*fixups is the softmax numerator up to a constant
        # e^{-c_past} per head, which cancels in the normalization.
        accp = ctx.enter_context(tc.tile_pool(name="accp", bufs=1, space="PSUM"))
        attn_ctx = ExitStack()
        nrm = attn_ctx.enter_context(tc.tile_pool(name="nrm", bufs=2))
        ptp = attn_ctx.enter_context(tc.tile_pool(name="ptp", bufs=6))

        denr_v = denr_d.ap()
        denc_v = denc_d.ap()

        pend = [None]

        def flush():
            if pend[0] is not None:
                pend[0]()
                pend[0] = None

        for qh in range(N_QH):
            w0 = qh * QH
            for pair in range(2):
                acc_a = accp.tile([DK + 1, QH], F32, tag="accv")
                acc_b = accp.tile([DK + 1, QH], F32, tag="accd")
                accs = [acc_a, acc_b]
                for kc in range(N_KC):
                    k0 = kc * 128
                    pt = ptp.tile([128, 2, QH], BF16, tag="pt")
                    for hh in range(2):
                        h = 2 * pair + hh
                        p0 = hh * 64
                        st = stp.tile([128, QH], F32, tag="st")
                        for half in range(2):
                            nc.tensor.matmul(
                                st[:, half * 512:(half + 1) * 512],
                                lhsT=k_sb[p0:p0 + 64, pair, k0:k0 + 128],
                                rhs=q_sb[p0:p0 + 64, pair,
                                         w0 + half * 512:w0 + (half + 1) * 512],
                                start=True, stop=True,
                                tile_position=(p0, 0),
                            )
                        fut_end = min(max(k0 - 128, w0), w0 + QH)
                        n_fut = fut_end - w0
                        if n_fut == QH:
                            # whole tile is in the far-future region: fold the
                            # constant multiplier into the exp bias.
                            nc.scalar.activation(out=pt[:, hh, :], in_=st,
                                                 func=EXP, scale=SCALE,
                                                 bias=fbias_sb[:, h:h + 1])
                        else:
                            nc.scalar.activation(out=pt[:, hh, :], in_=st,
                                                 func=EXP, scale=SCALE,
                                                 bias=pshift_sb[:, 0:1])
                    if n_fut < QH:
                        # one multiply fixes future + band for BOTH heads:
                        # col c needs bandx[:, h, FW + (c + w0 - k0 + 128)]
                        b_lo = max(k0 - 128, w0)
                        b_hi = min(k0 + 2 * 128, w0 + QH)
                        c_lo = 0 if n_fut > 0 else b_lo - w0
                        c_hi = b_hi - w0
                        if c_hi > c_lo:
                            i0 = FW + (c_lo + w0 - k0 + 128)
                            wdt = c_hi - c_lo
                            eng2 = nc.vector if wdt >= 640 else nc.gpsimd
                            eng2.tensor_mul(
                                out=pt[:, :, c_lo:c_hi],
                                in0=pt[:, :, c_lo:c_hi],
                                in1=bandx_sb[:, 2 * pair:2 * pair + 2, i0:i0 + wdt],
                            )
                    # software-pipelined PV: previous chunk's PV issues here
                    # so the PE never waits on this chunk's exp.
                    flush()

                    def mk(pt=pt, accs=accs, kc=kc, pair=pair):
                        def go():
                            for hh in range(2):
                                for sub in range(2):
                                    nc.tensor.matmul(
                                        accs[hh][:, sub * 512:(sub + 1) * 512],
                                        lhsT=v_sb[:, kc, 2 * pair + hh, :],
                                        rhs=pt[:, hh, sub * 512:(sub + 1) * 512],
                                        start=(kc == 0), stop=(kc == N_KC - 1),
                                    )
                        return go
                    pend[0] = mk()
                flush()

                # ---- normalize (qh, pair) ----
                r0 = (qh * 2 + pair) * 2
                den_sb = nrm.tile([1, 2, QH], F32, tag="den_sb")
                for hh in range(2):
                    nc.vector.tensor_copy(out=den_sb[:, hh, :],
                                          in_=accs[hh][DK:DK + 1, :])
                nc.sync.dma_start(out=denr_v[r0:r0 + 2, :], in_=den_sb)
                for hh in range(2):
                    nc.vector.tensor_copy(
                        out=ct_sb[hh * 64:hh * 64 + 64, pair, w0:w0 + QH],
                        in_=accs[hh][0:DK, :])
                # reciprocal in a transposed layout: dt[p, s, c] = den[r0+s][p*8+c]
                dt = nrm.tile([128, 2, 8], F32, tag="dt")
                gsrc = bass.AP(tensor=denr_v.tensor, offset=denr_v.offset + r0 * QH,
                               ap=[[8, 128], [QH, 2], [1, 8]])
                nc.sync.dma_start(out=dt, in_=gsrc)
                dt2 = nrm.tile([128, 2, 8], F32, tag="dt2")
                nc.vector.reciprocal(out=dt2, in_=dt)
                gdst = bass.AP(tensor=denc_v.tensor, offset=denc_v.offset + r0 * QH,
                               ap=[[8, 128], [QH, 2], [1, 8]])
                nc.sync.dma_start(out=gdst, in_=dt2)
                rbc = nrm.tile([128, QH], F32, tag="rbc")
                for hh in range(2):
                    bsrc = bass.AP(tensor=denc_v.tensor,
                                   offset=denc_v.offset + (r0 + hh) * QH,
                                   ap=[[0, 64], [1, QH]])
                    nc.sync.dma_start(out=rbc[hh * 64:hh * 64 + 64, :], in_=bsrc)
                nc.vector.tensor_mul(
                    out=ct_sb[:, pair, w0:w0 + QH],
                    in0=ct_sb[:, pair, w0:w0 + QH],
                    in1=rbc,
                )

        attn_ctx.close()

        # ---------- phase 3: Wo partial ----------
        for st_i in range(S // 128):
            pool = stp if st_i % 2 == 0 else accp
            ps = pool.tile([128, 1024], F32, tag="st" if st_i % 2 == 0 else "accv")
            o_sb = outp.tile([128, D], BF16, tag="o_sb")
            for j in range(2):
                for mt in range(2):
                    nc.tensor.matmul(
                        ps[:, mt * 512:(mt + 1) * 512],
                        lhsT=ct_sb[:, j, st_i * 128:(st_i + 1) * 128],
                        rhs=wo_sb[:, j, mt * 512:(mt + 1) * 512],
                        start=(j == 0), stop=(j == 1),
                    )
            if st_i % 2 == 0:
                nc.scalar.copy(out=o_sb, in_=ps)
            else:
                nc.vector.tensor_copy(out=o_sb, in_=ps)
            nc.sync.dma_start(out=out_d.ap()[st_i * 128:(st_i + 1) * 128, :], in_=o_sb)

    nc.compile()
    return nc


def make_core_inputs(x, Wq, bq, Wk, bk, Wv, bv, Wo, bo, rel_bias):
    """Host-side shard prep. Returns list of 8 in_maps."""
    x = np.asarray(x, np.float32)
    in_maps = []
    bf = ml_dtypes.bfloat16
    WqT = np.asarray(Wq, np.float32).T.astype(bf)
    WkT = np.asarray(Wk, np.float32).T.astype(bf)
    WvT = np.asarray(Wv, np.float32).T.astype(bf)
    WoT = np.asarray(Wo, np.float32).T.astype(bf)
    rel = np.asarray(rel_bias, np.float32)
    xt = [np.ascontiguousarray(x[b].T.astype(bf)) for b in range(B)]

    # band multiplier: [p, h_local, m] = exp(bias(q,k) - c_past), q-k = m-128-p
    p_i = np.arange(128)[:, None]
    m_i = np.arange(BAND)[None, :]
    delta = np.clip(m_i - 128 - p_i, -MAX_REL, MAX_REL) + MAX_REL  # [128, 384]

    for c in range(N_CORES):
        b = c // CORES_PER_BATCH
        g = c % CORES_PER_BATCH
        c0 = g * CL
        heads = np.arange(g * HEADS_PER_CORE, (g + 1) * HEADS_PER_CORE)

        bqk = np.empty((128, 4), np.float32)
        bqk[:, 0] = np.asarray(bq, np.float32)[c0:c0 + 128]
        bqk[:, 1] = np.asarray(bq, np.float32)[c0 + 128:c0 + 256]
        bqk[:, 2] = np.asarray(bk, np.float32)[c0:c0 + 128]
        bqk[:, 3] = np.asarray(bk, np.float32)[c0 + 128:c0 + 256]

        bandx = np.empty((128, HEADS_PER_CORE, FW + BAND), np.float32)
        fbias = np.empty((128, HEADS_PER_CORE), np.float32)
        for i, hg in enumerate(heads):
            c_past = rel[hg, 2 * MAX_REL]
            bandx[:, i, 0:FW] = np.exp(rel[hg, 0] - c_past)
            bandx[:, i, FW:] = np.exp(rel[hg][delta] - c_past)
            fbias[:, i] = rel[hg, 0] - c_past + PSHIFT
        in_maps.append({
            "xt": xt[b],
            "wqt": np.ascontiguousarray(WqT[:, c0:c0 + CL]),
            "wkt": np.ascontiguousarray(WkT[:, c0:c0 + CL]),
            "wvt": np.ascontiguousarray(WvT[:, c0:c0 + CL]),
            "wot": np.ascontiguousarray(WoT[c0:c0 + CL, :]),
            "bqk": bqk,
            "bandx": bandx.astype(bf),
            "fbias": fbias,
        })
    return in_maps


_NC_CACHE = {}


def get_program(**kw):
    key = tuple(sorted(kw.items()))
    if key not in _NC_CACHE:
        _NC_CACHE[key] = build_program(**kw)
    return _NC_CACHE[key]


def kernel(x, Wq, bq, Wk, bk, Wv, bv, Wo, bo, rel_bias):
    from concourse.bass_utils import run_bass_kernel_spmd

    nc = get_program()
    in_maps = make_core_inputs(x, Wq, bq, Wk, bk, Wv, bv, Wo, bo, rel_bias)
    res = run_bass_kernel_spmd(nc, in_maps, core_ids=list(range(N_CORES)))
    results = res.results

    Wo_np = np.asarray(Wo, np.float32)
    const = np.asarray(bv, np.float32) @ Wo_np.T + np.asarray(bo, np.float32)
    out = np.zeros((B, S, D), np.float32)
    for c in range(N_CORES):
        out[c // CORES_PER_BATCH] += results[c]["out_p"].astype(np.float32)
    out += const[None, None, :]
    return out
